# revision 16
# baseline (speedup 1.0000x reference)
"""Trainium2 Bass kernel for a 4-layer NeRF-style MLP.

    y = relu(relu(relu(x@W1.T+b1)@W2.T+b2)@W3.T+b3)@W4.T+b4
    x: [1048576, 6] fp32 -> y: [1048576, 4] fp32

Strategy: pure data parallel over 8 NeuronCores (131072 rows each).
On-device layout keeps features on SBUF partitions and rows on the free
dim, so every layer's PSUM output is directly the next layer's matmul
rhs — no transposes anywhere.

Per core, rows are processed in groups of 4 chunks x 512 rows:
  - layer 1 (K=6+1): the 4 chunks are packed into the four 32-row PE
    groups (tile_position row packing) and run concurrently; the bias is
    folded into the matmul via a constant ones-row in x (K=7).
  - layers 2/3 (K=128): one matmul per chunk, float32r (1 cycle/row).
  - layer 4 (M=4): col-packed via tile_position=(0,32g) so all 4 chunk
    outputs land in one PSUM bank -> single cheap eviction.
  - PSUM->SBUF evictions (fused bias+ReLU) are column-split between the
    Scalar (ACT) and Vector (DVE) engines to use both in parallel.
"""

import numpy as np

N = 1048576
CORES = 8
R = N // CORES            # rows per core
CHUNK = 512               # rows per matmul (one PSUM bank of fp32)
GPC = 4                   # chunks per group
GROUPS = R // (CHUNK * GPC)   # 64
GW = GPC * CHUNK          # 2048 columns per group
SA = 1216                 # ACT engine's column share of each eviction

_CACHE = {}


def _build():
    import concourse.bacc as bacc
    import concourse.mybir as mybir
    import concourse.tile as tile

    f32 = mybir.dt.float32
    f32r = mybir.dt.float32r
    Relu = mybir.ActivationFunctionType.Relu
    op_add = mybir.AluOpType.add
    op_max = mybir.AluOpType.max

    nc = bacc.Bacc("TRN2", target_bir_lowering=False, debug=False)

    xin = nc.dram_tensor("xin", [GROUPS, GPC, 7, CHUNK], f32r, kind="ExternalInput").ap()
    w1 = nc.dram_tensor("w1", [128, 128], f32r, kind="ExternalInput").ap()
    w2 = nc.dram_tensor("w2", [128, 128], f32r, kind="ExternalInput").ap()
    w3 = nc.dram_tensor("w3", [128, 128], f32r, kind="ExternalInput").ap()
    w4 = nc.dram_tensor("w4", [128, 4], f32r, kind="ExternalInput").ap()
    b2 = nc.dram_tensor("b2", [128, 1], f32, kind="ExternalInput").ap()
    b3 = nc.dram_tensor("b3", [128, 1], f32, kind="ExternalInput").ap()
    b4m = nc.dram_tensor("b4m", [128, 64], f32, kind="ExternalInput").ap()
    yout = nc.dram_tensor(
        "yout", [GROUPS, 128, 64], f32, kind="ExternalOutput"
    ).ap()

    with tile.TileContext(nc) as tc:
        with (
            tc.tile_pool(name="const", bufs=1) as cpool,
            tc.tile_pool(name="x", bufs=4) as xpool,
            tc.tile_pool(name="h", bufs=4) as hpool,
            tc.tile_pool(name="o", bufs=4) as opool,
            tc.tile_pool(name="psum", bufs=2, space="PSUM") as ppool,
        ):
            w1s = cpool.tile([128, 128], f32r, tag="w1")
            nc.sync.dma_start(out=w1s[:], in_=w1)
            w2s = cpool.tile([128, 128], f32r, tag="w2")
            nc.sync.dma_start(out=w2s[:], in_=w2)
            w3s = cpool.tile([128, 128], f32r, tag="w3")
            nc.sync.dma_start(out=w3s[:], in_=w3)
            w4s = cpool.tile([128, 4], f32r, tag="w4")
            nc.sync.dma_start(out=w4s[:], in_=w4)
            b2s = cpool.tile([128, 1], f32, tag="b2")
            nc.sync.dma_start(out=b2s[:], in_=b2)
            b3s = cpool.tile([128, 1], f32, tag="b3")
            nc.sync.dma_start(out=b3s[:], in_=b3)
            b4s = cpool.tile([128, 64], f32, tag="b4")
            nc.sync.dma_start(out=b4s[:], in_=b4m)

            w1r = w1s.rearrange("(a b) c -> a b c", b=32)

            for grp in range(GROUPS):
                xt = xpool.tile([128, CHUNK], f32r, tag="x")
                xtr = xt.rearrange("(a b) c -> a b c", b=32)
                # DMA APs only support a single leading partition dim, so
                # write each 7-partition row group with its own DMA
                for g in range(GPC):
                    nc.sync.dma_start(out=xtr[g, 0:7, :], in_=xin[grp, g])

                # layer 1: 4 chunks packed into the 4 PE row groups
                pt = ppool.tile([128, GW], f32, tag="pt")
                for g in range(GPC):
                    nc.tensor.matmul(
                        pt[:, g * CHUNK : (g + 1) * CHUNK],
                        lhsT=w1r[g, 0:7, :],
                        rhs=xtr[g, 0:7, :],
                        start=True,
                        stop=True,
                        tile_position=(32 * g, 0),
                    )
                h = hpool.tile([128, GW], f32r, tag="h")
                nc.scalar.activation(h[:, 0:SA], pt[:, 0:SA], Relu)
                nc.vector.tensor_scalar(
                    out=h[:, SA:GW],
                    in0=pt[:, SA:GW],
                    scalar1=0.0,
                    scalar2=None,
                    op0=op_max,
                )

                # layers 2 and 3
                for ws, bs in ((w2s, b2s), (w3s, b3s)):
                    pt = ppool.tile([128, GW], f32, tag="pt")
                    for g in range(GPC):
                        nc.tensor.matmul(
                            pt[:, g * CHUNK : (g + 1) * CHUNK],
                            lhsT=ws[:, :],
                            rhs=h[:, g * CHUNK : (g + 1) * CHUNK],
                            start=True,
                            stop=True,
                        )
                    hn = hpool.tile([128, GW], f32r, tag="h")
                    nc.scalar.activation(
                        hn[:, 0:SA], pt[:, 0:SA], Relu, bias=bs[:, 0:1]
                    )
                    nc.vector.tensor_scalar(
                        out=hn[:, SA:GW],
                        in0=pt[:, SA:GW],
                        scalar1=bs[:, 0:1],
                        scalar2=0.0,
                        op0=op_add,
                        op1=op_max,
                    )
                    h = hn

                # layer 4 transposed: h-slice is the stationary operand,
                # W4.T the moving one (N=4) -> output is [128 rows, 4 feats]
                # per 128-row slice; the whole group's output is a dense
                # [128, 64] PSUM block, so eviction is nearly free.
                pt = ppool.tile([128, 16 * 4], f32, tag="pt")
                for s in range(16):
                    nc.tensor.matmul(
                        pt[:, 4 * s : 4 * s + 4],
                        lhsT=h[:, 128 * s : 128 * (s + 1)],
                        rhs=w4s[:, :],
                        start=True,
                        stop=True,
                        skip_group_check=True,
                    )
                ot = opool.tile([128, 64], f32, tag="o")
                nc.vector.tensor_add(out=ot[:], in0=pt[:, 0:64], in1=b4s[:])
                nc.sync.dma_start(out=yout[grp], in_=ot[:])

    nc.compile()
    return nc


def _prep_in_maps(x, W1, b1, W2, b2, W3, b3, W4, b4):
    x = np.ascontiguousarray(np.asarray(x, dtype=np.float32))

    w1t = np.zeros((128, 128), np.float32)
    W1T = np.asarray(W1, np.float32).T  # [6, 128]
    for g in range(GPC):
        w1t[32 * g : 32 * g + 6, :] = W1T
        w1t[32 * g + 6, :] = np.asarray(b1, np.float32)
    w2t = np.ascontiguousarray(np.asarray(W2, np.float32).T)  # [128, 128]
    w3t = np.ascontiguousarray(np.asarray(W3, np.float32).T)
    w4t = np.ascontiguousarray(np.asarray(W4, np.float32).T)  # [128, 4]
    b2t = np.ascontiguousarray(np.asarray(b2, np.float32).reshape(128, 1))
    b3t = np.ascontiguousarray(np.asarray(b3, np.float32).reshape(128, 1))
    b4t = np.tile(np.asarray(b4, np.float32).reshape(1, 4), (128, 16))
    b4t = np.ascontiguousarray(b4t)  # [128, 64] = b4 tiled per 4-col slice

    in_maps = []
    for c in range(CORES):
        xc = x[c * R : (c + 1) * R]  # [R, 6]
        xr = xc.reshape(GROUPS, GPC, CHUNK, 6).transpose(0, 1, 3, 2)
        xi = np.empty((GROUPS, GPC, 7, CHUNK), np.float32)
        xi[:, :, 0:6, :] = xr
        xi[:, :, 6, :] = 1.0
        in_maps.append(
            {
                "xin": xi,
                "w1": w1t,
                "w2": w2t,
                "w3": w3t,
                "w4": w4t,
                "b2": b2t,
                "b3": b3t,
                "b4m": b4t,
            }
        )
    return in_maps


def _execute(in_maps, trace=False):
    from concourse.bass_utils import run_bass_kernel_spmd

    if "nc" not in _CACHE:
        _CACHE["nc"] = _build()
    return run_bass_kernel_spmd(
        _CACHE["nc"], in_maps, list(range(CORES)), trace=trace
    )


def bench(in_maps, iters=20):
    """Time repeated dispatches of the jitted sharded NEFF with
    device-resident inputs (no output-buffer donation, so buffers are
    reusable across calls). Returns per-iteration wall times in seconds.
    """
    import time

    import jax
    from jax.experimental.shard_map import shard_map
    from jax.sharding import Mesh, NamedSharding, PartitionSpec

    import concourse.mybir as mybir
    from concourse import bass2jax

    if "nc" not in _CACHE:
        _CACHE["nc"] = _build()
    nc = _CACHE["nc"]
    bass2jax.install_neuronx_cc_hook()

    in_names, out_names, out_avals = [], [], []
    for alloc in nc.m.functions[0].allocations:
        if not isinstance(alloc, mybir.MemoryLocationSet):
            continue
        name = alloc.memorylocations[0].name
        pid = nc.partition_id_tensor.name if nc.partition_id_tensor else None
        if alloc.kind == "ExternalInput":
            if name != pid:
                in_names.append(name)
        elif alloc.kind == "ExternalOutput":
            out_names.append(name)
            out_avals.append(
                jax.core.ShapedArray(
                    tuple(alloc.tensor_shape), mybir.dt.np(alloc.dtype)
                )
            )
    n_params = len(in_names)
    all_names = tuple(in_names + out_names)

    def _body(*args):
        operands = list(args)
        if nc.partition_id_tensor is not None:
            operands.append(bass2jax.partition_id_tensor())
        outs = bass2jax._bass_exec_p.bind(
            *operands,
            out_avals=tuple(out_avals),
            in_names=all_names
            + ((nc.partition_id_tensor.name,) if nc.partition_id_tensor else ()),
            out_names=tuple(out_names),
            lowering_input_output_aliases=(),
            sim_require_finite=True,
            sim_require_nnan=True,
            nc=nc,
        )
        return tuple(outs)

    devices = jax.devices()[:CORES]
    mesh = Mesh(np.asarray(devices), ("core",))
    in_specs = (PartitionSpec("core"),) * (n_params + len(out_names))
    out_specs = (PartitionSpec("core"),) * len(out_names)
    fn = jax.jit(
        shard_map(
            _body, mesh=mesh, in_specs=in_specs, out_specs=out_specs, check_rep=False
        ),
        keep_unused=True,
    )

    concat_in = [
        np.concatenate([np.asarray(in_maps[c][n]) for c in range(CORES)], axis=0)
        for n in in_names
    ]
    zeros = [
        np.zeros((CORES * av.shape[0], *av.shape[1:]), av.dtype) for av in out_avals
    ]
    sh = NamedSharding(mesh, PartitionSpec("core"))
    dev_in = [jax.device_put(a, sh) for a in concat_in]
    dev_zeros = [jax.device_put(z, sh) for z in zeros]

    out = fn(*dev_in, *dev_zeros)
    jax.block_until_ready(out)
    times = []
    for _ in range(iters):
        t0 = time.perf_counter()
        out = fn(*dev_in, *dev_zeros)
        jax.block_until_ready(out)
        times.append(time.perf_counter() - t0)
    return times


def kernel(**inputs):
    in_maps = _prep_in_maps(
        inputs["x"],
        inputs["W1"],
        inputs["b1"],
        inputs["W2"],
        inputs["b2"],
        inputs["W3"],
        inputs["b3"],
        inputs["W4"],
        inputs["b4"],
    )
    results = _execute(in_maps).results
    outs = []
    for c in range(CORES):
        # yout dims: (grp, p, (s, k)) -> row = grp*2048 + s*128 + p
        yo = np.asarray(results[c]["yout"]).reshape(GROUPS, 128, 16, 4)
        outs.append(yo.transpose(0, 2, 1, 3).reshape(R, 4))
    return np.ascontiguousarray(np.concatenate(outs, axis=0).astype(np.float32))
